# revision 7
# baseline (speedup 1.0000x reference)
"""Trainium2 Bass kernel for nn_Model2_65103114273350 (dense_cnn).

Pipeline (per image):
  conv3x3(18->32, SAME) + bias + relu -> global avg pool -> concat(pred)
  -> fc1(34->64) + relu -> fc2(64->9) + hierarchical mask -> softmax

Strategy: pure data parallel over batch (8 images per NeuronCore).

Conv: shift-matmul with dy packed into the contraction: K = 54 =
18ch x 3dy (the three row-shifted copies of x live on partitions
18*dy+c, built host-side), M = 32 out-channels, and the 3 dx taps
accumulate into PSUM via column-offset rhs views. The PE runs in
64x32 tile_position mode: 2 row-groups (image halves) x 4 col-groups
(pixel blocks) = 8 concurrent small matmuls, N = 448 (2 rows x 224).
x and conv weights are stored fp8e4m3 (weights pre-scaled by 16,
compensated exactly in bias and GAP fold) which doubles the PE's
byte-limited rhs streaming rate and halves DMA; GAP averaging over
50k pixels washes out the quantization noise (final rel err ~4e-5).

PSUM evacuation works on 2-round superrounds: each engine (ACT for
row-group 0, DVE scalar_tensor_tensor for row-group 1) processes 2
PSUM banks in one fused bias+relu+partial-GAP op via accum_out, which
amortizes the per-op fixed costs (172cyc ACT ramp + 284ns
READ_ACCUMULATOR) across 896 elements. (NB: tensor_scalar with
accum_out repurposes op1 as the reduction op - it does NOT apply op1
elementwise - so the STT form with a zeros in1 is required here.)
Matmuls within a superround run dx-outermost so the two rounds'
same-weight matmuls are adjacent, and a 12-matmul zero-input warmup
burst before the image loop flips the PE HAM throttle to full clock
while the first image is still streaming in. 2 tags x 2 bufs x 2 banks =
all 8 PSUM banks, double-buffered against the PE. A K=128 fold
matmul merges the 4 col-group partial sums and applies 1/(H*W). The
MLP head runs fully on-chip: biases AND the hierarchical softmax
mask (as idx * (row1-row0) + row0, magnitude -200) are folded into
the fc matmuls via homogeneous-coordinate rows.

x is host-packed into the exact SBUF partition layout (xprep), fp8,
loaded with 2 HWDGE (sync-queue) DMAs per half-image stripe (54
partitions x 12.6KB each, skipping the dead partitions 54-63).
HWDGE keeps descriptor generation off the GpSimd Q7 (the previous
18-partition SWDGE loads serialized 655ns/DMA of descriptor-gen plus
multi-us DRAINs there, starving the PE at image boundaries and
re-throttling it to 1.2GHz via HAM oscillation).
"""

import os
import sys

sys.path.insert(0, "/opt/trn_rl_repo")

import numpy as np
import ml_dtypes

import concourse.bass as bass
import concourse.tile as tile
from concourse import bacc, mybir
from concourse.bass_utils import run_bass_kernel_spmd

BF16 = ml_dtypes.float8_e4m3fn
F32 = mybir.dt.float32
BF = mybir.dt.float8e4
WSCALE = 16.0

B, C, H, W = 64, 18, 224, 224
O = 32
NCORES = 8
BB = B // NCORES
HP, WP = H + 2, W + 2
NG = 2                # PE row-groups (64-row tiling), K = 54 = 18ch x 3dy
GR = H // NG          # 112 output rows per group-stripe
KP = 54
RPR = 8               # output rows per stripe per round (4 col-tiles x 2 rows)
NROUNDS = GR // RPR   # 14
NSTRIPE = 4           # conv-bias replication factor over PSUM partitions
NL2 = 9

_VALID = np.full((2, NL2), -200.0, dtype=np.float32)
_VALID[0, 0:4] = 0.0
_VALID[1, 4:9] = 0.0

_cache: dict = {}


def build(n_images=BB, n_rounds=NROUNDS, debug=False):
    nc = bacc.Bacc(
        "TRN2",
        target_bir_lowering=False,
        debug=False,
        enable_asserts=False,
        num_devices=NCORES,
    )
    xprep = nc.dram_tensor("xprep", [BB, 2, 118, 56, WP], BF, kind="ExternalInput").ap()
    wpack = nc.dram_tensor("wpack", [3, KP, O], BF, kind="ExternalInput").ap()
    bias128 = nc.dram_tensor("bias128", [128, 1], F32, kind="ExternalInput").ap()
    foldw = nc.dram_tensor("foldw", [128, O], F32, kind="ExternalInput").ap()
    fc1w = nc.dram_tensor("fc1w", [35, 64], F32, kind="ExternalInput").ap()
    fc2w = nc.dram_tensor("fc2w", [67, NL2], F32, kind="ExternalInput").ap()
    pred3 = nc.dram_tensor("pred3", [3, BB], F32, kind="ExternalInput").ap()
    hrows = nc.dram_tensor("hrows", [3, BB], F32, kind="ExternalInput").ap()
    out_d = nc.dram_tensor("out", [BB, NL2], F32, kind="ExternalOutput").ap()
    if debug:
        gdbg = nc.dram_tensor("gdbg", [35, BB], F32, kind="ExternalOutput").ap()
        hdbg = nc.dram_tensor("hdbg", [65, BB], F32, kind="ExternalOutput").ap()

    AF = mybir.ActivationFunctionType
    ALU = mybir.AluOpType
    AX = mybir.AxisListType

    with tile.TileContext(nc) as tc:
        with (
            tc.tile_pool(name="consts", bufs=1) as consts,
            tc.tile_pool(name="persist", bufs=1) as persist,
        ):
            # conv weights (dy-packed K=54) replicated to the 2 PE row-groups.
            # consts ride the scalar (ACT) HWDGE queue so the sync queue is
            # free to issue the image-0 x loads immediately.
            wsb = consts.tile([128, 3, O], BF)
            wsrc = wpack.rearrange("s k m -> k s m")
            for g in range(NG):
                nc.scalar.dma_start(out=wsb[64 * g : 64 * g + KP, :, :], in_=wsrc)
            bias_sb = consts.tile([128, 1], F32)
            nc.scalar.dma_start(out=bias_sb[:, :], in_=bias128)
            fold_sb = consts.tile([128, O], F32)
            nc.scalar.dma_start(out=fold_sb[:, :], in_=foldw)
            fc1_sb = consts.tile([35, 64], F32)
            nc.scalar.dma_start(out=fc1_sb[:, :], in_=fc1w)
            fc2_sb = consts.tile([67, NL2], F32)
            nc.scalar.dma_start(out=fc2_sb[:, :], in_=fc2w)

            G = persist.tile([128, BB], F32)
            if n_images < BB:
                nc.vector.memset(G[:, :], 0.0)
            f_aug = persist.tile([35, BB], F32)
            nc.scalar.dma_start(out=f_aug[32:35, :], in_=pred3)
            h1_aug = persist.tile([67, BB], F32)
            nc.scalar.dma_start(out=h1_aug[64:67, :], in_=hrows)
            zt = persist.tile([128, 2, 2, W], F32)
            nc.vector.memset(zt[:, :, :, :], 0.0)
            wsc = persist.tile([128, 2, W], BF)
            nc.vector.memset(wsc[:, :, :], 0.0)
            warm = persist.tile([1, 1], F32)
            nc.vector.memset(warm[:, :], 0.0)
            nc.scalar.activation(warm[:, :], warm[:, :], AF.Exp)

            n_super = n_rounds // 2
            with (
                tc.tile_pool(name="xp", bufs=8) as xpool,
                tc.tile_pool(name="ps", bufs=2, space="PSUM") as pspool,
                tc.tile_pool(name="sl", bufs=2) as slpool,
            ):
                # HAM warmup: dummy matmuls on zeros keep the PE active while
                # the first image streams in, so real matmuls start at 2.4GHz.
                for w in range(12):
                    pw = pspool.tile(
                        [128, 2, 2, 256], F32, tag=f"b{w % 2}", name="pw"
                    )
                    nc.tensor.matmul(
                        pw[0:O, 0, :, 0:W],
                        wsb[0:KP, 0, :],
                        wsc[0:KP, :, :],
                        start=True,
                        stop=True,
                        tile_position=(0, 0),
                        skip_group_check=True,
                    )
                for i in range(n_images):
                    xts = []
                    for h in range(2):
                        xth = xpool.tile([128, 56, WP], BF, name=f"xt{h}", tag="xt")
                        xts.append(xth)
                        # 2 big HWDGE loads per half (skip dead partitions 54-63)
                        nc.sync.dma_start(
                            out=xth[0:KP, :, :], in_=xprep[i, h, 0:KP, :, :]
                        )
                        nc.sync.dma_start(
                            out=xth[64 : 64 + KP, :, :],
                            in_=xprep[i, h, 64 : 64 + KP, :, :],
                        )
                    st = slpool.tile([128, 16], F32)
                    for s in range(n_super):
                        # superround: 2 conv rounds -> 2 PSUM banks per group
                        pts = [
                            pspool.tile(
                                [128, 2, 2, 256], F32, tag=f"b{g}", name=f"pt{g}"
                            )
                            for g in range(NG)
                        ]
                        # dx outermost: consecutive same-weight matmuls per PE
                        # tile (rr=0,1) so the weight reload rate is halved
                        for dx in range(3):
                            for g in range(NG):
                                for c in range(4):
                                    for rr in range(2):
                                        t = 2 * s + rr
                                        xt = xts[t // 7]
                                        k0 = RPR * (t % 7) + 2 * c
                                        nc.tensor.matmul(
                                            pts[g][32 * c : 32 * c + O, rr, :, 0:W],
                                            wsb[64 * g : 64 * g + KP, dx, :],
                                            xt[64 * g : 64 * g + KP, k0 : k0 + 2, dx : dx + W],
                                            start=(dx == 0),
                                            stop=(dx == 2),
                                            tile_position=(64 * g, 32 * c),
                                            skip_group_check=True,
                                        )
                        # fused bias+relu+partial-GAP over both banks, split ACT/DVE
                        nc.scalar.activation(
                            pts[0][:, :, :, 0:W], pts[0][:, :, :, 0:W],
                            AF.Relu, bias=bias_sb[:, :],
                            accum_out=st[:, 2 * s : 2 * s + 1],
                        )
                        nc.vector.scalar_tensor_tensor(
                            out=pts[1][:, :, :, 0:W], in0=pts[1][:, :, :, 0:W],
                            scalar=bias_sb[:, :], in1=zt[:, :, :, :],
                            op0=ALU.add, op1=ALU.max,
                            accum_out=st[:, 2 * s + 1 : 2 * s + 2],
                        )
                    nc.vector.reduce_sum(
                        out=G[:, i : i + 1], in_=st[:, 0 : 2 * n_super], axis=AX.X
                    )

            with (
                tc.tile_pool(name="hps", bufs=1, space="PSUM") as hps,
                tc.tile_pool(name="mi", bufs=1) as mi,
            ):
                g_ps = hps.tile([O, BB], F32, tag="hp0")
                nc.tensor.matmul(g_ps[:, :], fold_sb[:, :], G[:, :], start=True, stop=True)
                nc.vector.tensor_copy(f_aug[0:O, :], g_ps[:, :])
                h1_ps = hps.tile([64, BB], F32, tag="hp1")
                nc.tensor.matmul(h1_ps[:, :], fc1_sb[:, :], f_aug[:, :], start=True, stop=True)
                nc.scalar.activation(h1_aug[0:64, :], h1_ps[:, :], AF.Relu)
                lg_ps = hps.tile([BB, NL2], F32, tag="hp2")
                nc.tensor.matmul(lg_ps[:, :], h1_aug[:, :], fc2_sb[:, :], start=True, stop=True)
                lg = mi.tile([BB, NL2], F32)
                mx = mi.tile([BB, 1], F32)
                nc.vector.reduce_max(out=mx[:, :], in_=lg_ps[:, :], axis=AX.X, negate=True)
                nc.scalar.activation(lg[:, :], lg_ps[:, :], AF.Exp, bias=mx[:, :])
                sm = mi.tile([BB, 1], F32)
                nc.vector.reduce_sum(out=sm[:, :], in_=lg[:, :], axis=AX.X)
                rc = mi.tile([BB, 1], F32)
                nc.vector.reciprocal(rc[:, :], sm[:, :])
                ot = mi.tile([BB, NL2], F32)
                nc.vector.tensor_scalar(
                    out=ot[:, :], in0=lg[:, :], scalar1=rc[:, :], scalar2=None,
                    op0=ALU.mult,
                )
                nc.sync.dma_start(out=out_d, in_=ot[:, :])
                if debug:
                    nc.sync.dma_start(out=gdbg, in_=f_aug[:, :])
                    nc.sync.dma_start(out=hdbg, in_=h1_aug[:, :])

    nc.compile()
    return nc


def prep_inputs(x, model1_pred, conv_w, conv_b, fc1_w, fc1_b, fc2_w, fc2_b):
    x = np.asarray(x, dtype=np.float32)
    model1_pred = np.asarray(model1_pred, dtype=np.float32)
    conv_w = np.asarray(conv_w, dtype=np.float32)
    conv_b = np.asarray(conv_b, dtype=np.float32)
    fc1_w = np.asarray(fc1_w, dtype=np.float32)
    fc1_b = np.asarray(fc1_b, dtype=np.float32)
    fc2_w = np.asarray(fc2_w, dtype=np.float32)
    fc2_b = np.asarray(fc2_b, dtype=np.float32)

    xpad = np.zeros((B, C, HP, WP), dtype=BF16)
    xpad[:, :, 1 : H + 1, 1 : W + 1] = x
    xprep = np.zeros((B, 2, 118, 56, WP), dtype=BF16)
    for h in range(2):
        for g in range(NG):
            for dy in range(3):
                p0 = 64 * g + 18 * dy
                r0 = GR * g + 56 * h + dy
                xprep[:, h, p0 : p0 + C] = xpad[:, :, r0 : r0 + 56, :]

    wpack = np.ascontiguousarray(
        conv_w.transpose(3, 2, 1, 0).reshape(3, KP, O) * WSCALE
    ).astype(BF16)
    bias128 = np.ascontiguousarray(
        np.tile(conv_b * WSCALE, NSTRIPE).reshape(128, 1).astype(np.float32)
    )

    foldw = np.zeros((128, O), dtype=np.float32)
    foldw[np.arange(128), np.arange(128) % O] = 1.0 / (H * W * WSCALE)

    fc1w_aug = np.zeros((35, 64), dtype=np.float32)
    fc1w_aug[:34] = fc1_w.T
    fc1w_aug[34] = fc1_b
    fc2w_aug = np.zeros((67, NL2), dtype=np.float32)
    fc2w_aug[:64] = fc2_w.T
    fc2w_aug[64] = fc2_b
    fc2w_aug[65] = _VALID[1] - _VALID[0]
    fc2w_aug[66] = _VALID[0]

    in_maps = []
    for i in range(NCORES):
        sl = slice(BB * i, BB * (i + 1))
        slq = slice(BB // 2 * i, BB // 2 * (i + 1))
        pred = model1_pred[sl]
        idx = np.argmax(pred, axis=1).astype(np.float32)
        ones = np.ones((1, BB), dtype=np.float32)
        pred3 = np.ascontiguousarray(np.vstack([pred.T, ones]))
        hrows = np.ascontiguousarray(np.vstack([ones, idx[None, :], ones]))
        in_maps.append(
            {
                "xprep": np.ascontiguousarray(xprep[sl]),
                "wpack": wpack,
                "bias128": bias128,
                "foldw": foldw,
                "fc1w": fc1w_aug,
                "fc2w": fc2w_aug,
                "pred3": pred3,
                "hrows": hrows,
            }
        )
    return in_maps


def _axon_ntff_hook():
    """ctypes NTFF-profiling hook into the axon PJRT plugin (the
    antenv.axon_hooks module is absent in this container, so wire it
    directly; recipe mirrors trn_agent_boot/trn_boot.py)."""
    import contextlib
    import ctypes

    lib = ctypes.CDLL("/opt/axon/libaxon_pjrt.so")
    if not hasattr(lib, "axon_start_nrt_profile"):
        return None
    lib.axon_start_nrt_profile.argtypes = [
        ctypes.POINTER(ctypes.c_int64),
        ctypes.c_size_t,
    ]
    lib.axon_start_nrt_profile.restype = ctypes.c_int64
    lib.axon_stop_nrt_profile.argtypes = [ctypes.c_char_p]
    lib.axon_stop_nrt_profile.restype = ctypes.c_int64

    @contextlib.contextmanager
    def _hook(output_dir, device_ids):
        import jax

        jax.devices()
        if device_ids:
            ids = (ctypes.c_int64 * len(device_ids))(*device_ids)
            rc = lib.axon_start_nrt_profile(ids, len(device_ids))
        else:
            rc = lib.axon_start_nrt_profile(None, 0)
        if rc != 0:
            raise RuntimeError(f"axon_start_nrt_profile rc={rc}")
        try:
            yield
        finally:
            n = lib.axon_stop_nrt_profile(str(output_dir).encode())
            print(f"profile: {n} file(s) written to {output_dir}")

    return _hook


def _exec_time_from_ntffs(tmpdir):
    """neuron-profile view each *_body* ntff against the largest neff;
    return max over cores of summary total_time (ns)."""
    import glob
    import json as _json
    import subprocess

    neffs = sorted(
        glob.glob(os.path.join(tmpdir, "*.neff")), key=os.path.getsize, reverse=True
    )
    ntffs = sorted(glob.glob(os.path.join(tmpdir, "*.ntff")))
    if not neffs or not ntffs:
        print(f"profile files missing in {tmpdir}: {os.listdir(tmpdir)}")
        return None, {}
    times = {}
    for ntff in ntffs:
        base = os.path.basename(ntff)
        jf = os.path.join(tmpdir, base + ".json")
        cmd = [
            "neuron-profile", "view", "--ignore-nc-buf-usage",
            "-s", ntff, "-n", neffs[0],
            "--output-format=json", f"--output-file={jf}",
            "--ignore-dma-trace",
        ]
        try:
            subprocess.check_call(cmd, cwd=tmpdir)
            with open(jf) as f:
                j = _json.load(f)
            times[base] = int(j["summary"][0]["total_time"] * 1e9)
        except Exception as e:  # noqa: BLE001
            print(f"neuron-profile failed for {base}: {e}")
    if not times:
        return None, {}
    return max(times.values()), times


def run(inputs, trace=False):
    if "nc" not in _cache:
        _cache["nc"] = build()
    nc = _cache["nc"]
    in_maps = prep_inputs(**inputs)
    if trace:
        import tempfile

        from concourse import bass2jax
        from concourse.bass_utils import BassKernelResults

        bass2jax.install_neuronx_cc_hook()
        hook = _axon_ntff_hook()
        tmpdir = tempfile.mkdtemp(prefix="ntff_")
        with hook(tmpdir, None):
            results = bass2jax.run_bass_via_pjrt(nc, in_maps, n_cores=NCORES)
        exec_ns, per_core = _exec_time_from_ntffs(tmpdir)
        print(f"per-ntff exec ns: {per_core}")
        print(f"profile dir: {tmpdir}")
        res = BassKernelResults(
            results=results,
            instructions_and_trace=None,
            profile_json=None,
            exec_time_ns=exec_ns,
        )
    else:
        res = run_bass_kernel_spmd(nc, in_maps, list(range(NCORES)), trace=False)
    out = np.concatenate(
        [np.asarray(res.results[i]["out"], dtype=np.float32) for i in range(NCORES)],
        axis=0,
    )
    return out, res


def kernel(**inputs) -> np.ndarray:
    out, _ = run(inputs, trace=False)
    return out



# revision 8
# speedup vs baseline: 1.1409x; 1.1409x over previous
"""Trainium2 Bass kernel for nn_Model2_65103114273350 (dense_cnn).

Pipeline (per image):
  conv3x3(18->32, SAME) + bias + relu -> global avg pool -> concat(pred)
  -> fc1(34->64) + relu -> fc2(64->9) + hierarchical mask -> softmax

Strategy: pure data parallel over batch (8 images per NeuronCore).

Conv: shift-matmul with dy packed into the contraction: K = 54 =
18ch x 3dy (the three row-shifted copies of x live on partitions
18*dy+c, built host-side), M = 32 out-channels, and the 3 dx taps
accumulate into PSUM via column-offset rhs views. The PE runs in
64x32 tile_position mode: 2 row-groups (image halves) x 4 col-groups
(pixel blocks) = 8 concurrent small matmuls, N = 448 (2 rows x 224).
x and conv weights are stored fp8e4m3 (weights pre-scaled by 16,
compensated exactly in bias and GAP fold) which doubles the PE's
byte-limited rhs streaming rate and halves DMA; GAP averaging over
50k pixels washes out the quantization noise (final rel err ~4e-5).

PSUM evacuation works on 2-round superrounds: each engine (ACT for
row-group 0, DVE scalar_tensor_tensor for row-group 1) processes 2
PSUM banks in one fused bias+relu+partial-GAP op via accum_out, which
amortizes the per-op fixed costs (172cyc ACT ramp + 284ns
READ_ACCUMULATOR) across 896 elements. (NB: tensor_scalar with
accum_out repurposes op1 as the reduction op - it does NOT apply op1
elementwise - so the STT form with a zeros in1 is required here.)
Matmuls within a superround run dx-outermost so the two rounds'
same-weight matmuls are adjacent, and a 12-matmul zero-input warmup
burst before the image loop flips the PE HAM throttle to full clock
while the first image is still streaming in. 2 tags x 2 bufs x 2 banks =
all 8 PSUM banks, double-buffered against the PE. A K=128 fold
matmul merges the 4 col-group partial sums and applies 1/(H*W). The
MLP head runs fully on-chip: biases AND the hierarchical softmax
mask (as idx * (row1-row0) + row0, magnitude -200) are folded into
the fc matmuls via homogeneous-coordinate rows.

x is host-packed into the exact SBUF partition layout (xprep), fp8,
loaded with 2 HWDGE (sync-queue) DMAs per half-image stripe (54
partitions x 12.6KB each, skipping the dead partitions 54-63).
HWDGE keeps descriptor generation off the GpSimd Q7 (the previous
18-partition SWDGE loads serialized 655ns/DMA of descriptor-gen plus
multi-us DRAINs there, starving the PE at image boundaries and
re-throttling it to 1.2GHz via HAM oscillation).
"""

import os
import sys

sys.path.insert(0, "/opt/trn_rl_repo")

import numpy as np
import ml_dtypes

import concourse.bass as bass
import concourse.tile as tile
from concourse import bacc, mybir
from concourse.bass_utils import run_bass_kernel_spmd

BF16 = ml_dtypes.float8_e4m3fn
F32 = mybir.dt.float32
BF = mybir.dt.float8e4
WSCALE = 16.0

B, C, H, W = 64, 18, 224, 224
O = 32
NCORES = 8
BB = B // NCORES
HP, WP = H + 2, W + 2
NG = 2                # PE row-groups (64-row tiling), K = 54 = 18ch x 3dy
GR = H // NG          # 112 output rows per group-stripe
KP = 54
RPR = 8               # output rows per stripe per round (4 col-tiles x 2 rows)
NROUNDS = GR // RPR   # 14
NSTRIPE = 4           # conv-bias replication factor over PSUM partitions
NL2 = 9

_VALID = np.full((2, NL2), -200.0, dtype=np.float32)
_VALID[0, 0:4] = 0.0
_VALID[1, 4:9] = 0.0

_cache: dict = {}


def build(n_images=BB, n_rounds=NROUNDS, debug=False):
    nc = bacc.Bacc(
        "TRN2",
        target_bir_lowering=False,
        debug=False,
        enable_asserts=False,
        num_devices=NCORES,
    )
    xprep = nc.dram_tensor("xprep", [BB, 2, 118, 56, WP], BF, kind="ExternalInput").ap()
    wpack = nc.dram_tensor("wpack", [3, KP, O], BF, kind="ExternalInput").ap()
    bias128 = nc.dram_tensor("bias128", [128, 1], F32, kind="ExternalInput").ap()
    foldw = nc.dram_tensor("foldw", [128, O], F32, kind="ExternalInput").ap()
    fc1w = nc.dram_tensor("fc1w", [35, 64], F32, kind="ExternalInput").ap()
    fc2w = nc.dram_tensor("fc2w", [67, NL2], F32, kind="ExternalInput").ap()
    pred3 = nc.dram_tensor("pred3", [3, BB], F32, kind="ExternalInput").ap()
    hrows = nc.dram_tensor("hrows", [3, BB], F32, kind="ExternalInput").ap()
    out_d = nc.dram_tensor("out", [BB, NL2], F32, kind="ExternalOutput").ap()
    if debug:
        gdbg = nc.dram_tensor("gdbg", [35, BB], F32, kind="ExternalOutput").ap()
        hdbg = nc.dram_tensor("hdbg", [65, BB], F32, kind="ExternalOutput").ap()

    AF = mybir.ActivationFunctionType
    ALU = mybir.AluOpType
    AX = mybir.AxisListType

    with tile.TileContext(nc) as tc:
        with (
            tc.tile_pool(name="consts", bufs=1) as consts,
            tc.tile_pool(name="persist", bufs=1) as persist,
        ):
            # conv weights (dy-packed K=54) replicated to the 2 PE row-groups.
            # consts ride the scalar (ACT) HWDGE queue so the sync queue is
            # free to issue the image-0 x loads immediately.
            wsb = consts.tile([128, 3, O], BF)
            wsrc = wpack.rearrange("s k m -> k s m")
            for g in range(NG):
                nc.scalar.dma_start(out=wsb[64 * g : 64 * g + KP, :, :], in_=wsrc)
            bias_sb = consts.tile([128, 1], F32)
            nc.scalar.dma_start(out=bias_sb[:, :], in_=bias128)
            fold_sb = consts.tile([128, O], F32)
            nc.scalar.dma_start(out=fold_sb[:, :], in_=foldw)
            fc1_sb = consts.tile([35, 64], F32)
            nc.scalar.dma_start(out=fc1_sb[:, :], in_=fc1w)
            fc2_sb = consts.tile([67, NL2], F32)
            nc.scalar.dma_start(out=fc2_sb[:, :], in_=fc2w)

            G = persist.tile([128, BB], F32)
            if n_images < BB:
                nc.vector.memset(G[:, :], 0.0)
            f_aug = persist.tile([35, BB], F32)
            nc.scalar.dma_start(out=f_aug[32:35, :], in_=pred3)
            h1_aug = persist.tile([67, BB], F32)
            nc.scalar.dma_start(out=h1_aug[64:67, :], in_=hrows)
            zt = persist.tile([128, 2, 2, W], F32)
            nc.vector.memset(zt[:, :, :, :], 0.0)
            wsc = persist.tile([128, 2, W], BF)
            nc.vector.memset(wsc[:, :, :], 0.0)
            warm = persist.tile([1, 1], F32)
            nc.vector.memset(warm[:, :], 0.0)
            nc.scalar.activation(warm[:, :], warm[:, :], AF.Exp)

            n_super = n_rounds // 2
            with (
                tc.tile_pool(name="xp", bufs=8) as xpool,
                tc.tile_pool(name="ps", bufs=2, space="PSUM") as pspool,
                tc.tile_pool(name="sl", bufs=2) as slpool,
            ):
                # HAM warmup: dummy matmuls on zeros keep the PE active while
                # the first image streams in, so real matmuls start at 2.4GHz.
                for w in range(12):
                    pw = pspool.tile(
                        [128, 2, 2, 256], F32, tag=f"b{w % 2}", name="pw"
                    )
                    nc.tensor.matmul(
                        pw[0:O, 0, :, 0:W],
                        wsb[0:KP, 0, :],
                        wsc[0:KP, :, :],
                        start=True,
                        stop=True,
                        tile_position=(0, 0),
                        skip_group_check=True,
                    )
                for i in range(n_images):
                    xts = []
                    for h in range(2):
                        xth = xpool.tile([128, 56, WP], BF, name=f"xt{h}", tag="xt")
                        xts.append(xth)
                        # 2 big SWDGE loads per half (skip dead partitions
                        # 54-63). SWDGE chunks descriptors into <=4KB packets,
                        # so the SDMA engines can interleave the PE's IRAM
                        # refill fetches; HWDGE emits one multi-us packet per
                        # engine which blocks refills and stalls the PE.
                        nc.gpsimd.dma_start(
                            out=xth[0:KP, :, :], in_=xprep[i, h, 0:KP, :, :]
                        )
                        nc.gpsimd.dma_start(
                            out=xth[64 : 64 + KP, :, :],
                            in_=xprep[i, h, 64 : 64 + KP, :, :],
                        )
                    st = slpool.tile([128, 16], F32)
                    for s in range(n_super):
                        # superround: 2 conv rounds -> 2 PSUM banks per group
                        pts = [
                            pspool.tile(
                                [128, 2, 2, 256], F32, tag=f"b{g}", name=f"pt{g}"
                            )
                            for g in range(NG)
                        ]
                        # dx outermost: consecutive same-weight matmuls per PE
                        # tile (rr=0,1) so the weight reload rate is halved
                        for dx in range(3):
                            for g in range(NG):
                                for c in range(4):
                                    for rr in range(2):
                                        t = 2 * s + rr
                                        xt = xts[t // 7]
                                        k0 = RPR * (t % 7) + 2 * c
                                        nc.tensor.matmul(
                                            pts[g][32 * c : 32 * c + O, rr, :, 0:W],
                                            wsb[64 * g : 64 * g + KP, dx, :],
                                            xt[64 * g : 64 * g + KP, k0 : k0 + 2, dx : dx + W],
                                            start=(dx == 0),
                                            stop=(dx == 2),
                                            tile_position=(64 * g, 32 * c),
                                            skip_group_check=True,
                                        )
                        # fused bias+relu+partial-GAP over both banks, split ACT/DVE
                        nc.scalar.activation(
                            pts[0][:, :, :, 0:W], pts[0][:, :, :, 0:W],
                            AF.Relu, bias=bias_sb[:, :],
                            accum_out=st[:, 2 * s : 2 * s + 1],
                        )
                        nc.vector.scalar_tensor_tensor(
                            out=pts[1][:, :, :, 0:W], in0=pts[1][:, :, :, 0:W],
                            scalar=bias_sb[:, :], in1=zt[:, :, :, :],
                            op0=ALU.add, op1=ALU.max,
                            accum_out=st[:, 2 * s + 1 : 2 * s + 2],
                        )
                    nc.vector.reduce_sum(
                        out=G[:, i : i + 1], in_=st[:, 0 : 2 * n_super], axis=AX.X
                    )

            with (
                tc.tile_pool(name="hps", bufs=1, space="PSUM") as hps,
                tc.tile_pool(name="mi", bufs=1) as mi,
            ):
                g_ps = hps.tile([O, BB], F32, tag="hp0")
                nc.tensor.matmul(g_ps[:, :], fold_sb[:, :], G[:, :], start=True, stop=True)
                nc.vector.tensor_copy(f_aug[0:O, :], g_ps[:, :])
                h1_ps = hps.tile([64, BB], F32, tag="hp1")
                nc.tensor.matmul(h1_ps[:, :], fc1_sb[:, :], f_aug[:, :], start=True, stop=True)
                nc.scalar.activation(h1_aug[0:64, :], h1_ps[:, :], AF.Relu)
                lg_ps = hps.tile([BB, NL2], F32, tag="hp2")
                nc.tensor.matmul(lg_ps[:, :], h1_aug[:, :], fc2_sb[:, :], start=True, stop=True)
                lg = mi.tile([BB, NL2], F32)
                mx = mi.tile([BB, 1], F32)
                nc.vector.reduce_max(out=mx[:, :], in_=lg_ps[:, :], axis=AX.X, negate=True)
                nc.scalar.activation(lg[:, :], lg_ps[:, :], AF.Exp, bias=mx[:, :])
                sm = mi.tile([BB, 1], F32)
                nc.vector.reduce_sum(out=sm[:, :], in_=lg[:, :], axis=AX.X)
                rc = mi.tile([BB, 1], F32)
                nc.vector.reciprocal(rc[:, :], sm[:, :])
                ot = mi.tile([BB, NL2], F32)
                nc.vector.tensor_scalar(
                    out=ot[:, :], in0=lg[:, :], scalar1=rc[:, :], scalar2=None,
                    op0=ALU.mult,
                )
                nc.sync.dma_start(out=out_d, in_=ot[:, :])
                if debug:
                    nc.sync.dma_start(out=gdbg, in_=f_aug[:, :])
                    nc.sync.dma_start(out=hdbg, in_=h1_aug[:, :])

    nc.compile()
    return nc


def prep_inputs(x, model1_pred, conv_w, conv_b, fc1_w, fc1_b, fc2_w, fc2_b):
    x = np.asarray(x, dtype=np.float32)
    model1_pred = np.asarray(model1_pred, dtype=np.float32)
    conv_w = np.asarray(conv_w, dtype=np.float32)
    conv_b = np.asarray(conv_b, dtype=np.float32)
    fc1_w = np.asarray(fc1_w, dtype=np.float32)
    fc1_b = np.asarray(fc1_b, dtype=np.float32)
    fc2_w = np.asarray(fc2_w, dtype=np.float32)
    fc2_b = np.asarray(fc2_b, dtype=np.float32)

    xpad = np.zeros((B, C, HP, WP), dtype=BF16)
    xpad[:, :, 1 : H + 1, 1 : W + 1] = x
    xprep = np.zeros((B, 2, 118, 56, WP), dtype=BF16)
    for h in range(2):
        for g in range(NG):
            for dy in range(3):
                p0 = 64 * g + 18 * dy
                r0 = GR * g + 56 * h + dy
                xprep[:, h, p0 : p0 + C] = xpad[:, :, r0 : r0 + 56, :]

    wpack = np.ascontiguousarray(
        conv_w.transpose(3, 2, 1, 0).reshape(3, KP, O) * WSCALE
    ).astype(BF16)
    bias128 = np.ascontiguousarray(
        np.tile(conv_b * WSCALE, NSTRIPE).reshape(128, 1).astype(np.float32)
    )

    foldw = np.zeros((128, O), dtype=np.float32)
    foldw[np.arange(128), np.arange(128) % O] = 1.0 / (H * W * WSCALE)

    fc1w_aug = np.zeros((35, 64), dtype=np.float32)
    fc1w_aug[:34] = fc1_w.T
    fc1w_aug[34] = fc1_b
    fc2w_aug = np.zeros((67, NL2), dtype=np.float32)
    fc2w_aug[:64] = fc2_w.T
    fc2w_aug[64] = fc2_b
    fc2w_aug[65] = _VALID[1] - _VALID[0]
    fc2w_aug[66] = _VALID[0]

    in_maps = []
    for i in range(NCORES):
        sl = slice(BB * i, BB * (i + 1))
        slq = slice(BB // 2 * i, BB // 2 * (i + 1))
        pred = model1_pred[sl]
        idx = np.argmax(pred, axis=1).astype(np.float32)
        ones = np.ones((1, BB), dtype=np.float32)
        pred3 = np.ascontiguousarray(np.vstack([pred.T, ones]))
        hrows = np.ascontiguousarray(np.vstack([ones, idx[None, :], ones]))
        in_maps.append(
            {
                "xprep": np.ascontiguousarray(xprep[sl]),
                "wpack": wpack,
                "bias128": bias128,
                "foldw": foldw,
                "fc1w": fc1w_aug,
                "fc2w": fc2w_aug,
                "pred3": pred3,
                "hrows": hrows,
            }
        )
    return in_maps


def _axon_ntff_hook():
    """ctypes NTFF-profiling hook into the axon PJRT plugin (the
    antenv.axon_hooks module is absent in this container, so wire it
    directly; recipe mirrors trn_agent_boot/trn_boot.py)."""
    import contextlib
    import ctypes

    lib = ctypes.CDLL("/opt/axon/libaxon_pjrt.so")
    if not hasattr(lib, "axon_start_nrt_profile"):
        return None
    lib.axon_start_nrt_profile.argtypes = [
        ctypes.POINTER(ctypes.c_int64),
        ctypes.c_size_t,
    ]
    lib.axon_start_nrt_profile.restype = ctypes.c_int64
    lib.axon_stop_nrt_profile.argtypes = [ctypes.c_char_p]
    lib.axon_stop_nrt_profile.restype = ctypes.c_int64

    @contextlib.contextmanager
    def _hook(output_dir, device_ids):
        import jax

        jax.devices()
        if device_ids:
            ids = (ctypes.c_int64 * len(device_ids))(*device_ids)
            rc = lib.axon_start_nrt_profile(ids, len(device_ids))
        else:
            rc = lib.axon_start_nrt_profile(None, 0)
        if rc != 0:
            raise RuntimeError(f"axon_start_nrt_profile rc={rc}")
        try:
            yield
        finally:
            n = lib.axon_stop_nrt_profile(str(output_dir).encode())
            print(f"profile: {n} file(s) written to {output_dir}")

    return _hook


def _exec_time_from_ntffs(tmpdir):
    """neuron-profile view each *_body* ntff against the largest neff;
    return max over cores of summary total_time (ns)."""
    import glob
    import json as _json
    import subprocess

    neffs = sorted(
        glob.glob(os.path.join(tmpdir, "*.neff")), key=os.path.getsize, reverse=True
    )
    ntffs = sorted(glob.glob(os.path.join(tmpdir, "*.ntff")))
    if not neffs or not ntffs:
        print(f"profile files missing in {tmpdir}: {os.listdir(tmpdir)}")
        return None, {}
    times = {}
    for ntff in ntffs:
        base = os.path.basename(ntff)
        jf = os.path.join(tmpdir, base + ".json")
        cmd = [
            "neuron-profile", "view", "--ignore-nc-buf-usage",
            "-s", ntff, "-n", neffs[0],
            "--output-format=json", f"--output-file={jf}",
            "--ignore-dma-trace",
        ]
        try:
            subprocess.check_call(cmd, cwd=tmpdir)
            with open(jf) as f:
                j = _json.load(f)
            times[base] = int(j["summary"][0]["total_time"] * 1e9)
        except Exception as e:  # noqa: BLE001
            print(f"neuron-profile failed for {base}: {e}")
    if not times:
        return None, {}
    return max(times.values()), times


def run(inputs, trace=False):
    if "nc" not in _cache:
        _cache["nc"] = build()
    nc = _cache["nc"]
    in_maps = prep_inputs(**inputs)
    if trace:
        import tempfile

        from concourse import bass2jax
        from concourse.bass_utils import BassKernelResults

        bass2jax.install_neuronx_cc_hook()
        hook = _axon_ntff_hook()
        tmpdir = tempfile.mkdtemp(prefix="ntff_")
        with hook(tmpdir, None):
            results = bass2jax.run_bass_via_pjrt(nc, in_maps, n_cores=NCORES)
        exec_ns, per_core = _exec_time_from_ntffs(tmpdir)
        print(f"per-ntff exec ns: {per_core}")
        print(f"profile dir: {tmpdir}")
        res = BassKernelResults(
            results=results,
            instructions_and_trace=None,
            profile_json=None,
            exec_time_ns=exec_ns,
        )
    else:
        res = run_bass_kernel_spmd(nc, in_maps, list(range(NCORES)), trace=False)
    out = np.concatenate(
        [np.asarray(res.results[i]["out"], dtype=np.float32) for i in range(NCORES)],
        axis=0,
    )
    return out, res


def kernel(**inputs) -> np.ndarray:
    out, _ = run(inputs, trace=False)
    return out

